# revision 10
# baseline (speedup 1.0000x reference)
"""GCN layer (mean/max message passing + 2-layer MLP) on 8 Trainium2 cores.

Strategy (node partitioning, data-parallel over nodes):
  - Nodes sorted by in-degree, dealt to 8 cores in 128-node blocks
    (stripe j -> global blocks 8j..8j+7, core c gets block (j, c)).
  - Per stripe an ELL slot table of width D_j (= max degree in stripe, ~= mean
    degree because of the sort) holds per-node source ids; pad slots point at a
    zeros row so they contribute 0 to the sum and (thanks to a +128 shift that
    makes all real values positive) never win the max.
  - The gather uses the production multi-index `dma_gather` (int16 indices).
    To fit 50001 rows into int16, stripes are grouped so each group's distinct
    source set is < 32k; each (core, group) gets a private deduped gather table
    built host-side. One dma_gather per 128-node block: slot-major index list
    k = s*128+p -> out[p, s, :].
  - On device per block: gather -> DVE sum/max reduces over slots ->
    mean = sum * (1/cnt) (per-partition scalar) -> PE transposes to
    feature-major -> MLP matmuls (relu via ACT with per-partition bias b0' =
    b0 - 128*w0.sum(1), which also cancels the +128 shift) -> residual add
    (resid = x0 + b1 precomputed) -> store node-major rows.
"""

import sys

sys.path.insert(0, "/opt/trn_rl_repo")

import numpy as np

P = 128
NCORES = 8
GROUP_LIMIT = 31000
_CACHE = {}


def _build_program(NBLK, NGRP, TBL_ROWS, D_list, grp_of_blk, Dmax, ioffs, IDXTOT):
    import concourse.bacc as bacc
    import concourse.mybir as mybir
    import concourse.tile as tile
    from concourse.masks import make_identity

    F2 = 256  # 2 * F (both batches per row)
    # dma_gather waits for ~2*(num_idxs/16+1)*16 descriptor-ring slots up
    # front; the carveout is dynamic_dma_scratch_size/16 slots (per-partition
    # SBUF bytes). 32KB -> 2048 slots; gathers are chunked to <=896 indices
    # (1824 slots) per op to fit.
    nc = bacc.Bacc(
        "TRN2", target_bir_lowering=False, dynamic_dma_scratch_size=32768
    )
    dt = mybir.dt
    tbl_d = nc.dram_tensor("tbl", [NGRP, TBL_ROWS, F2], dt.float32, kind="ExternalInput")
    idx_d = nc.dram_tensor("ellidx", [P, IDXTOT], dt.int16, kind="ExternalInput")
    inv_d = nc.dram_tensor("invc", [P, NBLK], dt.float32, kind="ExternalInput")
    resid_d = nc.dram_tensor("resid", [NBLK, P, F2], dt.float32, kind="ExternalInput")
    w0a_d = nc.dram_tensor("w0aT", [P, P], dt.float32, kind="ExternalInput")
    w0b_d = nc.dram_tensor("w0bT", [P, P], dt.float32, kind="ExternalInput")
    w1_d = nc.dram_tensor("w1T", [P, P], dt.float32, kind="ExternalInput")
    b0_d = nc.dram_tensor("b0p", [P, 1], dt.float32, kind="ExternalInput")
    out_d = nc.dram_tensor("out", [NBLK, P, F2], dt.float32, kind="ExternalOutput")

    with tile.TileContext(nc) as tc:
        with (
            tc.tile_pool(name="const", bufs=1) as cpool,
            tc.tile_pool(name="work", bufs=3) as wpool,
            tc.tile_pool(name="psum", bufs=2, space="PSUM") as ppool,
        ):
            ident = cpool.tile([P, P], dt.float32)
            make_identity(nc, ident[:])
            w0a_t = cpool.tile([P, P], dt.float32)
            w0b_t = cpool.tile([P, P], dt.float32)
            w1_t = cpool.tile([P, P], dt.float32)
            b0_t = cpool.tile([P, 1], dt.float32)
            inv_t = cpool.tile([P, NBLK], dt.float32)
            idx_t = cpool.tile([P, IDXTOT], dt.int16)
            nc.sync.dma_start(out=w0a_t[:], in_=w0a_d[:])
            nc.sync.dma_start(out=w0b_t[:], in_=w0b_d[:])
            nc.sync.dma_start(out=w1_t[:], in_=w1_d[:])
            nc.sync.dma_start(out=b0_t[:], in_=b0_d[:])
            nc.sync.dma_start(out=inv_t[:], in_=inv_d[:])
            nc.sync.dma_start(out=idx_t[:], in_=idx_d[:])

            nreg_cache = {}

            def nreg(v):
                if v not in nreg_cache:
                    nreg_cache[v] = nc.gpsimd.to_reg(v)
                return nreg_cache[v]

            CHUNK = 7  # slots per gather op (<=896 indices)
            for j in range(NBLK):
                D = D_list[j]
                gt = wpool.tile([P, Dmax, F2], dt.float32, tag="gbuf")
                for s0 in range(0, D, CHUNK):
                    sd = min(CHUNK, D - s0)
                    nc.gpsimd.dma_gather(
                        out_ap=gt[:, s0 : s0 + sd, :],
                        in_ap=tbl_d[grp_of_blk[j]],
                        idxs_ap=idx_t[
                            :, ioffs[j] + 8 * s0 : ioffs[j] + 8 * (s0 + sd)
                        ],
                        num_idxs=P * sd,
                        num_idxs_reg=nreg(P * sd),
                        elem_size=F2,
                    )

                sum_t = wpool.tile([P, F2], dt.float32, tag="sum")
                max_t = wpool.tile([P, F2], dt.float32, tag="max")
                gv = gt[:, :D, :].rearrange("p d f -> p f d")
                nc.vector.tensor_reduce(
                    out=sum_t[:], in_=gv, axis=mybir.AxisListType.X,
                    op=mybir.AluOpType.add,
                )
                nc.vector.tensor_reduce(
                    out=max_t[:], in_=gv, axis=mybir.AxisListType.X,
                    op=mybir.AluOpType.max,
                )
                mean_t = wpool.tile([P, F2], dt.float32, tag="mean")
                nc.vector.tensor_scalar_mul(mean_t[:], sum_t[:], inv_t[:, j : j + 1])

                # feature-major transposes: [node, f] -> [f, node], both batches
                statT_ps = ppool.tile([P, 2 * F2], dt.float32, tag="statT")
                for b in range(2):
                    nc.tensor.transpose(
                        out=statT_ps[:, b * P : (b + 1) * P],
                        in_=mean_t[:, b * P : (b + 1) * P],
                        identity=ident[:],
                    )
                    nc.tensor.transpose(
                        out=statT_ps[:, F2 + b * P : F2 + (b + 1) * P],
                        in_=max_t[:, b * P : (b + 1) * P],
                        identity=ident[:],
                    )
                statT_s = wpool.tile([P, 2 * F2], dt.float32, tag="statTs")
                nc.scalar.copy(statT_s[:, :F2], statT_ps[:, :F2])
                nc.scalar.copy(statT_s[:, F2:], statT_ps[:, F2:])

                # h^T[o, n] = relu(w0a^T mean^T + w0b^T amax^T + b0')
                h_ps = ppool.tile([P, F2], dt.float32, tag="h")
                for b in range(2):
                    nc.tensor.matmul(
                        out=h_ps[:, b * P : (b + 1) * P],
                        lhsT=w0a_t[:],
                        rhs=statT_s[:, b * P : (b + 1) * P],
                        start=True, stop=False,
                    )
                    nc.tensor.matmul(
                        out=h_ps[:, b * P : (b + 1) * P],
                        lhsT=w0b_t[:],
                        rhs=statT_s[:, F2 + b * P : F2 + (b + 1) * P],
                        start=False, stop=True,
                    )
                hT_s = wpool.tile([P, F2], dt.float32, tag="hT")
                nc.scalar.activation(
                    hT_s[:], h_ps[:], mybir.ActivationFunctionType.Relu,
                    bias=b0_t[:],
                )

                # out[n, o2] = h^T.T @ w1^T + resid
                o_ps = ppool.tile([P, F2], dt.float32, tag="o")
                for b in range(2):
                    nc.tensor.matmul(
                        out=o_ps[:, b * P : (b + 1) * P],
                        lhsT=hT_s[:, b * P : (b + 1) * P],
                        rhs=w1_t[:],
                        start=True, stop=True,
                    )
                resid_t = wpool.tile([P, F2], dt.float32, tag="resid")
                nc.sync.dma_start(out=resid_t[:], in_=resid_d[j])
                out_s = wpool.tile([P, F2], dt.float32, tag="outs")
                nc.vector.tensor_add(out=out_s[:], in0=o_ps[:], in1=resid_t[:])
                nc.sync.dma_start(out=out_d[j], in_=out_s[:])

    nc.compile()
    return nc


def _prepare(x0, dst, src, w0, b0, w1, b1):
    B, N, F = x0.shape
    E = dst.shape[0]
    F2 = 2 * F

    deg = np.bincount(dst, minlength=N).astype(np.int64)
    deg_eff = np.maximum(deg, 1)
    order = np.argsort(deg, kind="stable")  # nodes by degree asc

    # ELL table [N, Dmax]: pads marked with N
    Dmax = int(deg.max())
    eperm = np.argsort(dst, kind="stable")
    dst_s = dst[eperm]
    src_s = src[eperm]
    starts = np.zeros(N + 1, np.int64)
    starts[1:] = np.cumsum(deg)
    ell = np.full((N, Dmax), N, np.int32)
    pos = np.arange(E, dtype=np.int64) - starts[dst_s]
    ell[dst_s, pos] = src_s
    zdeg = np.where(deg == 0)[0]
    ell[zdeg, 0] = zdeg  # deg-0 keeps own value via self slot

    # rank layout: NBLK stripes of 1024 ranks (8 cores x 128 partitions)
    NBLK = (N + NCORES * P - 1) // (NCORES * P)
    NR = NBLK * NCORES * P
    NDUM = NR - N
    node_of_rank = np.full(NR, -1, np.int64)
    node_of_rank[NDUM:] = order
    deg_of_rank = np.zeros(NR, np.int64)
    deg_of_rank[NDUM:] = deg_eff[order]

    D_list = [
        max(int(deg_of_rank[j * NCORES * P : (j + 1) * NCORES * P].max()), 1)
        for j in range(NBLK)
    ]
    offs = np.zeros(NBLK, np.int64)
    offs[1:] = np.cumsum(D_list)[:-1]
    ioffs = [int(8 * offs[j]) for j in range(NBLK)]
    IDXTOT = int(8 * (offs[-1] + D_list[-1]))

    rank_grid = np.arange(NR).reshape(NBLK, NCORES, P)
    ell_ranked = np.full((NR, Dmax), N, np.int32)
    real = node_of_rank >= 0
    ell_ranked[real] = ell[node_of_rank[real]]

    # per-(core, block) unique source sets -> greedy stripe groups with
    # bounded distinct-source count (shared group boundaries across cores)
    uniq = [[None] * NBLK for _ in range(NCORES)]
    for c in range(NCORES):
        for j in range(NBLK):
            e = ell_ranked[rank_grid[j, c], : D_list[j]]
            uniq[c][j] = np.unique(e[e < N])
    groups = []  # list of (start, end_exclusive)
    cur = [np.empty(0, np.int32)] * NCORES
    start = 0
    for j in range(NBLK):
        cand = [np.union1d(cur[c], uniq[c][j]) for c in range(NCORES)]
        if j > start and max(len(a) for a in cand) > GROUP_LIMIT:
            groups.append((start, j))
            start = j
            cur = [uniq[c][j] for c in range(NCORES)]
        else:
            cur = cand
    groups.append((start, NBLK))
    NGRP = len(groups)
    grp_of_blk = np.zeros(NBLK, np.int64)
    for g, (s, e) in enumerate(groups):
        grp_of_blk[s:e] = g

    # distinct source sets per (core, group)
    tvals = [[None] * NGRP for _ in range(NCORES)]
    for c in range(NCORES):
        for g, (s, e) in enumerate(groups):
            arrs = [uniq[c][j] for j in range(s, e)]
            tv = np.unique(np.concatenate(arrs)) if arrs else np.empty(0, np.int32)
            assert len(tv) + 1 < 32768
            tvals[c][g] = tv
    TBL_ROWS = max(len(tvals[c][g]) for c in range(NCORES) for g in range(NGRP)) + 1

    # shifted node rows: row n = [x0[0,n]+128, x0[1,n]+128]
    x0t = np.ascontiguousarray(x0.transpose(1, 0, 2).reshape(N, F2))
    x0shift = x0t + 128.0

    w0aT = np.ascontiguousarray(w0[:, :F].T).astype(np.float32)
    w0bT = np.ascontiguousarray(w0[:, F:].T).astype(np.float32)
    w1T = np.ascontiguousarray(w1.T).astype(np.float32)
    b0p = (b0 - 128.0 * w0.sum(axis=1)).astype(np.float32).reshape(P, 1)

    inv_ranked = np.ones(NR, np.float32)
    inv_ranked[real] = (1.0 / deg_eff[node_of_rank[real]]).astype(np.float32)
    resid_ranked = np.zeros((NR, F2), np.float32)
    b1t = np.tile(b1, B).astype(np.float32)
    resid_ranked[real] = x0t[node_of_rank[real]] + b1t

    karange = np.arange(P * Dmax)
    in_maps = []
    for c in range(NCORES):
        ranks_c = rank_grid[:, c, :]  # [NBLK, P]
        tbl = np.zeros((NGRP, TBL_ROWS, F2), np.float32)
        for g in range(NGRP):
            tv = tvals[c][g]
            tbl[g, 1 : 1 + len(tv)] = x0shift[tv]
        idxrep = np.zeros((P, IDXTOT), np.int16)
        for j in range(NBLK):
            D = D_list[j]
            tv = tvals[c][grp_of_blk[j]]
            e = ell_ranked[ranks_c[j], :D]  # [P, D], pads = N
            local = np.where(e < N, np.searchsorted(tv, np.minimum(e, N - 1)) + 1, 0)
            flat = local.T.ravel()  # k = s*128 + p
            nidx = P * D
            ka = karange[:nidx]
            wrapped = np.zeros((16, 8 * D), np.int16)
            wrapped[ka % 16, ka // 16] = flat.astype(np.int16)
            idxrep[:, ioffs[j] : ioffs[j] + 8 * D] = np.tile(wrapped, (8, 1))
        invcT = np.ascontiguousarray(inv_ranked[ranks_c].T)  # [P, NBLK]
        resid_c = np.ascontiguousarray(resid_ranked[ranks_c])  # [NBLK, P, F2]
        in_maps.append(
            {
                "tbl": tbl,
                "ellidx": idxrep,
                "invc": invcT,
                "resid": resid_c,
                "w0aT": w0aT,
                "w0bT": w0bT,
                "w1T": w1T,
                "b0p": b0p,
            }
        )

    meta = dict(
        N=N, B=B, F=F, F2=F2, NBLK=NBLK, NR=NR, NDUM=NDUM,
        D_list=D_list, Dmax=Dmax, ioffs=ioffs, IDXTOT=IDXTOT,
        NGRP=NGRP, TBL_ROWS=TBL_ROWS, grp_of_blk=grp_of_blk,
        node_of_rank=node_of_rank, rank_grid=rank_grid,
    )
    return in_maps, meta


def kernel(x0, dst, src, w0, b0, w1, b1, _run_results=None):
    x0 = np.asarray(x0, np.float32)
    dst = np.asarray(dst, np.int32)
    src = np.asarray(src, np.int32)
    w0 = np.asarray(w0, np.float32)
    b0 = np.asarray(b0, np.float32)
    w1 = np.asarray(w1, np.float32)
    b1 = np.asarray(b1, np.float32)

    in_maps, meta = _prepare(x0, dst, src, w0, b0, w1, b1)
    N, B, F, F2 = meta["N"], meta["B"], meta["F"], meta["F2"]
    NBLK = meta["NBLK"]

    key = (
        N, meta["Dmax"], meta["IDXTOT"], meta["NGRP"], meta["TBL_ROWS"],
        tuple(meta["D_list"]), tuple(meta["grp_of_blk"]),
    )
    if key not in _CACHE:
        _CACHE[key] = _build_program(
            NBLK, meta["NGRP"], meta["TBL_ROWS"], meta["D_list"],
            meta["grp_of_blk"], meta["Dmax"], meta["ioffs"], meta["IDXTOT"],
        )
    nc = _CACHE[key]

    if _run_results is None:
        from concourse.bass_utils import run_bass_kernel_spmd

        res = run_bass_kernel_spmd(nc, in_maps, list(range(NCORES)))
        results = res.results
    else:
        results = _run_results(nc, in_maps)

    # assemble: out_ranked[rank] rows -> nodes
    out_ranked = np.empty((meta["NR"], F2), np.float32)
    for c in range(NCORES):
        out_c = results[c]["out"]  # [NBLK, P, F2]
        out_ranked[meta["rank_grid"][:, c, :]] = out_c
    node_of_rank = meta["node_of_rank"]
    real = node_of_rank >= 0
    out_t = np.empty((N, F2), np.float32)
    out_t[node_of_rank[real]] = out_ranked[real]
    return np.ascontiguousarray(
        out_t.reshape(N, B, F).transpose(1, 0, 2)
    ).astype(np.float32)
